# revision 17
# baseline (speedup 1.0000x reference)
"""Trainium2 Bass kernel for nn_AttentionBlock (B=32, C=256, H=W=32).

Computation (per batch element b), algebraically restructured:
    scores^T = x^T (M x) (+ wh-row term; cancels per-column consts in softmax)
                 with M = Wk^T Wq   [bilinear form: one 256-ch G' = Mx
                 projection replaces the 512-ch Q,K pair]
    E        = exp(scores^T / 16)      # no max-subtraction (safe range)
    Z[hw]    = sum_HW E[HW, hw]
    U^T      = x^T (w_out w_v)^T       # w_out folded into V (linearity)
    out      = (U^T-contract E) * (1/Z) + b_f + x

Sharding: data-parallel over batch, 4 batch elements per core, 8 cores,
weights replicated. No on-device transpose anywhere.

v4 performance structure:
  - Z partial sums: exp tiles pair-summed on GPSIMD (idle engine),
    quad-summed on DVE; only 4 ones-matmuls (2048 PE cyc) on the PE
    instead of 16 (8192).
  - 1/Z via DVE reciprocal_approx_fast on the [1,1024] PSUM row; its
    issue is deferred into the NEXT batch's fused pass (after the m==2
    eviction) so the 1.2us DVE op never head-of-line blocks the fused
    evictions.  Broadcast across partitions via one DRAM round-trip on
    the sync queue (zero PE cost, latency fully hidden).
  - finalize (normalize-mul on DVE, residual-add split GPSIMD/DVE,
    per-chunk store) for batch b is issued after batch b+1's fused pass,
    when its rb has long arrived - nothing gates the PE.
  - b_in/b_out are zero for this model (spec fill), so wh == 0 and
    b_f == 0: G eviction is a plain cast split ACT/DVE per chunk, and
    the residual add is a plain tensor_add.  Generic nonzero-bias
    fallback kept (DVE scalar_tensor_tensor).
  - last batch: F runs n-outer with no PSUM eviction (normalize reads F
    straight out of PSUM); both Z-halves' reciprocal/bf16-rank-1 chains
    run while the n1 matmuls do, so the tail is just two muls + adds +
    stores.
  - x loads bf16-only in 4 chunks; 10 warm-up matmuls bridge the initial
    DMA wait so the PE HAM clock-gate never re-throttles after warming.

Accuracy: bf16 operands, fp32 accumulate; bf16 residual + partial-sum
tree + bf16 r-broadcast give rel err ~2e-3 (tolerance 2e-2).

Env knobs:
    ATTN_TRACE   = 0 | 1   (collect NTFF trace via run_bass_kernel_spmd)
"""

import math
import os
import sys

import numpy as np

B, C, HW = 32, 256, 1024
NCORES = 8
BPC = B // NCORES  # batch elements per core
CH_T = C // 128  # channel partition tiles (2)
HW_T = HW // 128  # spatial partition tiles (8)
NF = 512  # matmul free-dim chunk (one PSUM bank of fp32)
N_CH = HW // NF  # free-dim chunks (2)

_cache = {}
last_results = None  # BassKernelResults of the most recent run (for test.py)


def _ensure_path():
    for p in ("/opt/trn_rl_repo",):
        if os.path.isdir(p) and p not in sys.path:
            sys.path.append(p)


def _build(zero_bias: bool):
    """Build + compile the Bass kernel once. Returns the compiled nc."""
    _ensure_path()
    import concourse.bass as bass
    import concourse.mybir as mybir
    import concourse.tile as tile
    from concourse import bacc

    f32 = mybir.dt.float32
    bf16 = mybir.dt.bfloat16
    Alu = mybir.AluOpType
    Act = mybir.ActivationFunctionType

    cdt = bf16

    nc = bacc.Bacc(
        "TRN2", target_bir_lowering=False, debug=False, enable_asserts=False
    )

    xm_d = nc.dram_tensor("xm", [BPC, C, HW], cdt, kind="ExternalInput")
    mT_d = nc.dram_tensor("mT", [C, C], cdt, kind="ExternalInput")
    wu_d = nc.dram_tensor("wuT", [C, C], cdt, kind="ExternalInput")
    bias_d = nc.dram_tensor("bias", [128, 4], f32, kind="ExternalInput")
    out_d = nc.dram_tensor("out", [BPC, C, HW], f32, kind="ExternalOutput")

    def ns(n):
        return slice(n * NF, (n + 1) * NF)

    with tile.TileContext(nc) as tc:
        with (
            tc.tile_pool(name="const", bufs=1) as const,
            tc.tile_pool(name="xp", bufs=3) as xp,
            tc.tile_pool(name="qkp", bufs=2) as qkp,
            tc.tile_pool(name="up", bufs=2) as up,
            tc.tile_pool(name="ep", bufs=2) as ep,
            tc.tile_pool(name="wp", bufs=5) as wp,
            tc.tile_pool(name="rp", bufs=2) as rp,
            tc.tile_pool(name="op", bufs=6) as op_,
            tc.tile_pool(name="tp", bufs=4) as tp,
            tc.tile_pool(name="ps", bufs=6, space="PSUM") as ps,
            tc.tile_pool(name="zp", bufs=1, space="PSUM") as zp,
            tc.tile_pool(name="rd", bufs=2, space="DRAM") as rd,
        ):

            # HAM warm-up: keep the PE busy until the first x chunk lands so
            # the clock gate warms once and never re-throttles. The memset
            # runs on GPSIMD (earliest-booting engine) so the matmuls start
            # ~1.5us sooner; 6 of them end right as the data arrives.
            warm_sb = const.tile([128, NF], cdt, tag="warm")
            nc.gpsimd.memset(warm_sb[:], 0.0)
            warm_ps = [
                ps.tile([128, NF], f32, tag="ps", name="warm_ps")
                for _ in range(2)
            ]
            for i in range(6):
                nc.tensor.matmul(
                    warm_ps[i % 2][:],
                    warm_sb[:, 0:128],
                    warm_sb[:],
                    start=True,
                    stop=True,
                )

            def load_x(b):
                # 4 chunks so the first G matmul can start after ~1/4 of the
                # load; sync queue.
                x_mm = xp.tile([128, CH_T, HW], cdt, tag="xmm", name="x_mm")
                for ci in range(CH_T):
                    for n in range(N_CH):
                        nc.sync.dma_start(
                            out=x_mm[:, ci, ns(n)],
                            in_=xm_d[b, ci * 128 : (ci + 1) * 128, ns(n)],
                        )
                return x_mm

            x_next = load_x(0)

            # ---- weights / constants (loaded once, single DMA each) ----
            mT_sb = const.tile([128, CH_T, C], cdt, tag="mT")
            nc.scalar.dma_start(
                out=mT_sb[:], in_=mT_d[:].rearrange("(t p) f -> p t f", p=128)
            )
            wu_sb = const.tile([128, CH_T, C], cdt, tag="wu")
            nc.scalar.dma_start(
                out=wu_sb[:], in_=wu_d[:].rearrange("(t p) f -> p t f", p=128)
            )
            bias_sb = const.tile([128, 4], f32, tag="bias")
            nc.scalar.dma_start(out=bias_sb[:], in_=bias_d[:])
            bf_sb = bias_sb[:, 0:2]
            wh_sb = bias_sb[:, 2:4]  # h = x^T Wk^T bq (zero for this model)
            ones_col = const.tile([128, 1], cdt, tag="ones")
            nc.vector.memset(ones_col[:], 1.0)
            ones_row = const.tile([1, 128], cdt, tag="onesr")
            nc.vector.memset(ones_row[:], 1.0)

            def norm_add_store(b0, m, n, fo, rb, x0, add_eng="gpsimd"):
                # one [128, NF] chunk: t = fo*rb; o = t (+ b_f) + x; store
                t_sb = tp.tile([128, NF], f32, tag="t", name="t_sb")
                nc.vector.tensor_mul(t_sb[:], fo, rb)
                o_sb = op_.tile([128, NF], f32, tag="o", name="o_sb")
                if zero_bias:
                    eng = nc.gpsimd if add_eng == "gpsimd" else nc.vector
                    eng.tensor_add(o_sb[:], t_sb[:], x0[:, m, ns(n)])
                else:
                    nc.vector.scalar_tensor_tensor(
                        o_sb[:],
                        t_sb[:],
                        bf_sb[:, m : m + 1],
                        x0[:, m, ns(n)],
                        op0=Alu.add,
                        op1=Alu.add,
                    )
                nc.sync.dma_start(
                    out=out_d[b0, m * 128 : (m + 1) * 128, ns(n)],
                    in_=o_sb[:],
                )

            def finalize(fin):
                # normalize + bias + residual + store for a completed batch;
                # deferred until after the NEXT batch's fused pass so rb
                # (DMA broadcast) has long arrived
                b0, fo_t, rb_box, x0 = fin
                rb_sb = rb_box[0]
                for m in range(CH_T):
                    for n in range(N_CH):
                        norm_add_store(
                            b0, m, n, fo_t[m][n], rb_sb[:, ns(n)], x0
                        )

            def g_evict(dst, src, m):
                # zero-bias: plain ACT cast (the G window is ACT-idle and
                # this keeps the fused-pass DVE queue clear); else DVE
                # bias-add
                if not zero_bias:
                    nc.vector.tensor_scalar_add(dst, src, wh_sb[:, m : m + 1])
                    return
                nc.scalar.copy(dst, src)

            pending = None  # (b, fo_all, rb_box, x_mm) awaiting finalize
            pending_z = None  # (z_ps, rb_box) awaiting recip + broadcast
            for b in range(BPC):
                x_mm = x_next
                if b + 1 < BPC:
                    x_next = load_x(b + 1)
                last = b + 1 == BPC

                # ---- G = M x (+ wh), M = Wk^T Wq ----
                g_sb = qkp.tile([128, CH_T, HW], cdt, tag="g")
                for m in range(CH_T):
                    pst = [
                        ps.tile([128, NF], f32, tag="ps", name="ps")
                        for _ in range(N_CH)
                    ]
                    for ci in range(CH_T):
                        lhsT = mT_sb[:, ci, m * 128 : (m + 1) * 128]
                        for n in range(N_CH):
                            nc.tensor.matmul(
                                pst[n][:],
                                lhsT,
                                x_mm[:, ci, ns(n)],
                                start=(ci == 0),
                                stop=(ci == CH_T - 1),
                            )
                    for n in range(N_CH):
                        g_evict(g_sb[:, m, ns(n)], pst[n][:], m)

                # ---- fused x-stationary pass: each x[ci, m*128:...] slice is
                # loaded once as the stationary operand and serves the U^T
                # and S'^T matmuls ----
                uT_sb = up.tile([128, HW_T, C], cdt, tag="uT")
                e_sb = ep.tile([128, HW_T, HW], cdt, tag="e")
                p_tiles = []
                for m in range(HW_T):
                    pstU = ps.tile([128, C], f32, tag="ps", name="pstU")
                    pstS = [
                        ps.tile([128, NF], f32, tag="ps", name="ps")
                        for _ in range(N_CH)
                    ]
                    for ci in range(CH_T):
                        lhsT = x_mm[:, ci, m * 128 : (m + 1) * 128]
                        nc.tensor.matmul(
                            pstU[:],
                            lhsT,
                            wu_sb[:, ci, :],
                            start=(ci == 0),
                            stop=(ci == CH_T - 1),
                        )
                        for n in range(N_CH):
                            nc.tensor.matmul(
                                pstS[n][:],
                                lhsT,
                                g_sb[:, ci, ns(n)],
                                start=(ci == 0),
                                stop=(ci == CH_T - 1),
                            )
                    nc.vector.tensor_copy(uT_sb[:, m, :], pstU[:])
                    for n in range(N_CH):
                        nc.scalar.activation(
                            e_sb[:, m, ns(n)],
                            pstS[n][:],
                            Act.Exp,
                            scale=1.0 / math.sqrt(C),
                        )
                    if m == 2 and pending_z is not None:
                        # previous batch's Z reduce + 1/Z + broadcast:
                        # issued here so the ones-matmuls never wait on the
                        # GPSIMD pair-sums and the 1.2us DVE reciprocal sits
                        # behind the first evictions instead of blocking them
                        q_prev, rb_box = pending_z
                        pending_z = None
                        z_ps = zp.tile([1, HW], f32, tag="z", name="z_ps")
                        for n_ in range(N_CH):
                            for qi in range(2):
                                nc.tensor.matmul(
                                    z_ps[:, ns(n_)],
                                    ones_col[:],
                                    q_prev[qi][:, ns(n_)],
                                    start=(qi == 0),
                                    stop=(qi == 1),
                                )
                        r_row = rp.tile([1, HW], f32, tag="r")
                        nc.vector.reciprocal_approx_fast(r_row[:], z_ps[:])
                        r_dram = rd.tile([1, HW], f32, tag="rdram")
                        nc.sync.dma_start(out=r_dram[:], in_=r_row[:])
                        rb_sb = rp.tile([128, HW], f32, tag="rb")
                        r_ap = r_dram[:]
                        r_bc = bass.AP(
                            tensor=r_ap.tensor,
                            offset=r_ap.offset,
                            ap=[[0, 128], [1, HW]],
                        )
                        nc.sync.dma_start(out=rb_sb[:], in_=r_bc)
                        rb_box.append(rb_sb)
                    if m % 2 == 1:
                        # Z partial: pair-sum exp tiles; GPSIMD normally
                        # (idle engine), DVE per n-half for the last batch
                        # (lower latency -> Z chains run inside the F phase)
                        p_sb = wp.tile([128, HW], cdt, tag="p", name="p_sb")
                        if last:
                            for n in range(N_CH):
                                nc.vector.tensor_add(
                                    p_sb[:, ns(n)],
                                    e_sb[:, m - 1, ns(n)],
                                    e_sb[:, m, ns(n)],
                                )
                        else:
                            nc.gpsimd.tensor_add(
                                p_sb[:], e_sb[:, m - 1, :], e_sb[:, m, :]
                            )
                        p_tiles.append(p_sb)
                if not last:
                    q_tiles = []
                    for qi in range(2):
                        q_sb = wp.tile(
                            [128, HW], cdt, tag="q", name="q_sb", bufs=3
                        )
                        nc.vector.tensor_add(
                            q_sb[:], p_tiles[2 * qi][:], p_tiles[2 * qi + 1][:]
                        )
                        q_tiles.append(q_sb)

                if pending is not None:
                    finalize(pending)
                    pending = None

                def z_matmuls(z_ps, n):
                    # last batch: reduce straight over the 4 pair tiles
                    # (no q-level adds on the critical chain)
                    for pi in range(4):
                        nc.tensor.matmul(
                            z_ps[:, ns(n)],
                            ones_col[:],
                            p_tiles[pi][:, ns(n)],
                            start=(pi == 0),
                            stop=(pi == 3),
                        )

                # ---- F[o,hw] = sum_HW uT[HW,o] E[HW,hw] ----
                def f_kloop(pst, m, n):
                    for k in range(HW_T):
                        lhsT = uT_sb[:, k, m * 128 : (m + 1) * 128]
                        nc.tensor.matmul(
                            pst[:],
                            lhsT,
                            e_sb[:, k, ns(n)],
                            start=(k == 0),
                            stop=(k == HW_T - 1),
                        )

                if not last:
                    fo_all = []
                    # all 4 PSUM tiles upfront: the m1 k-loop must not wait
                    # on m0's eviction (pool ring reuse distance)
                    pst_all = [
                        [
                            ps.tile([128, NF], f32, tag="ps", name="ps")
                            for _ in range(N_CH)
                        ]
                        for _ in range(CH_T)
                    ]
                    for m in range(CH_T):
                        pst = pst_all[m]
                        for k in range(HW_T):
                            lhsT = uT_sb[:, k, m * 128 : (m + 1) * 128]
                            for n in range(N_CH):
                                nc.tensor.matmul(
                                    pst[n][:],
                                    lhsT,
                                    e_sb[:, k, ns(n)],
                                    start=(k == 0),
                                    stop=(k == HW_T - 1),
                                )
                        fo_n = []
                        for n in range(N_CH):
                            fo_sb = tp.tile(
                                [128, NF], f32, tag="fo", name="fo_sb", bufs=8
                            )
                            # split evictions ACT/DVE so neither engine's
                            # backlog gates PSUM recycling
                            if n == 0:
                                nc.scalar.copy(fo_sb[:], pst[n][:])
                            else:
                                nc.vector.tensor_copy(fo_sb[:], pst[n][:])
                            fo_n.append(fo_sb[:])
                        fo_all.append(fo_n)

                    # Z reduce + recip + broadcast all issued next batch
                    # (fused m==2), where the quad sums have long settled
                    rb_box = []
                    pending_z = (q_tiles, rb_box)
                    pending = (b, fo_all, rb_box, x_mm)
                else:
                    # ---- last batch: n-outer F, no PSUM eviction
                    # (normalize reads F straight from PSUM); both Z chains
                    # run while the n1 matmuls do ----
                    pst = [
                        [
                            ps.tile([128, NF], f32, tag="ps", name="ps")
                            for n in range(N_CH)
                        ]
                        for m in range(CH_T)
                    ]
                    z_ps = zp.tile([1, HW], f32, tag="z", name="z_ps")
                    r_row = rp.tile([1, HW], f32, tag="r")
                    r16 = rp.tile([1, HW], cdt, tag="r16")
                    rb_sb = rp.tile([128, N_CH, NF], f32, tag="rbl")
                    # F n0
                    for m in range(CH_T):
                        f_kloop(pst[m][0], m, 0)
                    # both Z halves -> reciprocal -> bf16 rows
                    for n in range(N_CH):
                        z_matmuls(z_ps, n)
                    for n in range(N_CH):
                        nc.vector.reciprocal_approx_fast(
                            r_row[:, ns(n)], z_ps[:, ns(n)]
                        )
                        nc.scalar.copy(r16[:, ns(n)], r_row[:, ns(n)])
                    # F n1 m0, then rank-1 broadcasts (r16 ready by then)
                    f_kloop(pst[0][1], 0, 1)
                    rb_ps = [
                        ps.tile([128, NF], f32, tag="ps", name="rb_ps")
                        for _ in range(N_CH)
                    ]
                    for n in range(N_CH):
                        nc.tensor.matmul(
                            rb_ps[n][:],
                            ones_row[:],
                            r16[:, ns(n)],
                            start=True,
                            stop=True,
                        )
                        nc.scalar.copy(rb_sb[:, n, :], rb_ps[n][:])
                    # n0 normalize+store overlaps the n1 m1 k-loop
                    for m in range(CH_T):
                        norm_add_store(
                            b,
                            m,
                            0,
                            pst[m][0][:],
                            rb_sb[:, 0, :],
                            x_mm,
                            add_eng="gpsimd" if m == 0 else "vector",
                        )
                    f_kloop(pst[1][1], 1, 1)
                    for m in range(CH_T):
                        norm_add_store(
                            b,
                            m,
                            1,
                            pst[m][1][:],
                            rb_sb[:, 1, :],
                            x_mm,
                            add_eng="gpsimd" if m == 0 else "vector",
                        )

            if pending is not None:
                finalize(pending)

    nc.compile()
    return nc


def kernel(x, w_in, b_in, w_out, b_out):
    global last_results
    _ensure_path()
    import ml_dtypes
    from concourse import bass_utils

    trace = os.environ.get("ATTN_TRACE", "0") == "1"

    x = np.ascontiguousarray(np.asarray(x, dtype=np.float32))
    w_in = np.asarray(w_in, dtype=np.float32)
    b_in = np.asarray(b_in, dtype=np.float32)
    w_out = np.asarray(w_out, dtype=np.float32)
    b_out = np.asarray(b_out, dtype=np.float32)

    # host-side weight prep (tiny)
    w_q = w_in[:C]
    w_k = w_in[C : 2 * C]
    w_v = w_in[2 * C :]
    b_q = b_in[:C]
    b_v = b_in[2 * C :]
    w_u = w_out @ w_v  # fold output projection into V
    # scores via bilinear form: S = x^T (Wk^T Wq) x + h(HW) (+ per-column
    # terms that cancel in softmax); lhsT of the G-projection is M^T
    m_mat = w_k.T @ w_q  # [256, 256]
    w_h = w_k.T @ b_q  # h = x^T Wk^T bq, added during the G eviction
    b_f = w_out @ b_v + b_out  # [256]
    zero_bias = bool(np.all(w_h == 0.0) and np.all(b_f == 0.0))

    key = ("k", zero_bias)
    if key not in _cache:
        _cache[key] = _build(zero_bias)
    nc = _cache[key]

    np_cdt = ml_dtypes.bfloat16
    mT = np.ascontiguousarray(m_mat.T.astype(np_cdt))
    wuT = np.ascontiguousarray(w_u.T.astype(np_cdt))  # [256, 256]
    bias = np.stack(
        [b_f[:128], b_f[128:], w_h[:128], w_h[128:]], axis=1
    )  # [128, 4]
    bias = np.ascontiguousarray(bias.astype(np.float32))

    xr = x.reshape(B, C, HW)
    xm = xr.astype(np_cdt)
    in_maps = []
    for c in range(NCORES):
        m = {
            "xm": np.ascontiguousarray(xm[c * BPC : (c + 1) * BPC]),
            "mT": mT,
            "wuT": wuT,
            "bias": bias,
        }
        in_maps.append(m)

    res = bass_utils.run_bass_kernel_spmd(
        nc, in_maps, core_ids=list(range(NCORES)), trace=trace
    )
    last_results = res

    out = np.concatenate([res.results[i]["out"] for i in range(NCORES)], axis=0)
    return out.reshape(B, C, 32, 32).astype(np.float32)
